# revision 1
# baseline (speedup 1.0000x reference)
"""Trainium2 Bass kernel for nn_Colar_static (retrieval_knn).

Sharding: data-parallel over batch B=2048 across 8 NeuronCores (256 rows each).
Static exemplar banks and weights are precomputed/reshaped on host and
replicated to all cores.

Per-core pipeline (all layouts keep batch in the matmul FREE dim or on
partitions as needed; j = flattened (class, exemplar) = 21*32 = 672):
  1. kvT[o,b]   = WkvT-blocks^T @ xT          (PE, bf16, K=2048)
  2. kT,vT      = psum evict (+bias, relu for v)  (ACT)
  3. sumsq[b]   = ones-matmul over kT^2       (DVE square + PE)
     rinv[b]    = 1/sqrt(sumsq)               (ACT sqrt + DVE recip)
  4. dot[b,j]   = kT-blocks^T @ Ekn_mat       (PE, K=1024)
  5. e = exp(rinv*dot)  (ACT, per-partition scale; cos in [-1,1] so no
     max-subtraction is needed for softmax stability)
  6. S,num      = blockwise reduces over 32-exemplar groups (DVE)
     t = num/S; g = exp(t); fw = g/G; c = fw/S  (class softmax; the scalar
     bias bw cancels in softmax)
  7. u[b,j] = e * c (block-broadcast)         (DVE)
  8. uT = PE-transpose(u)                     (PE + copies)
  9. fE_T[o,b]  = A_mat-blocks^T @ uT         (PE, K=672), relu evict
 10. outT[cls,b]= WoutT-blocks^T @ [relu(vT); relu(fE_T)]  (PE, K=2048)
 11. out = psum + bout -> DMA                 (DVE)

Host gathers the 8 [21,256] results into [2048, 21, 1].
"""

import numpy as np
import ml_dtypes

import concourse.bass as bass
import concourse.bacc as bacc
import concourse.mybir as mybir
import concourse.tile as tile
from concourse.bass_utils import run_bass_kernel_spmd

AF = mybir.ActivationFunctionType
BF = mybir.dt.bfloat16
F32 = mybir.dt.float32
bf16 = ml_dtypes.bfloat16

# Problem constants (hardcoded; kernel.py must be self-contained)
B, T, CIN, CH, M, NCLS = 2048, 8, 2048, 1024, 32, 21
NCORES = 8
BL = B // NCORES          # 256 batch rows per core
J = NCLS * M              # 672
P = 128
KB = CIN // P             # 16 contraction blocks for kv
OB = 2 * CH // P          # 16 output-channel blocks for kv
KHB = CH // P             # 8 blocks of k/v half
JBS = [P] * 5 + [J - 5 * P]   # j blocks: 5x128 + 32
NB = BL // P              # 2 batch chunks of 128


def build_nc(debug=False, repeat=1):
    nc = bacc.Bacc("TRN2", target_bir_lowering=False, debug=debug,
                   num_devices=NCORES)

    # all inputs are shipped in the exact per-partition SBUF layout so every
    # DMA is a plain [128, N]-contiguous copy (max DMA efficiency)
    xt_e = nc.dram_tensor("xt", [P, KB * BL], BF, kind="ExternalInput")
    wkv_e = nc.dram_tensor("wkv", [OB, P, KB * P], BF, kind="ExternalInput")
    ekn_e = nc.dram_tensor("ekn", [P, KHB * J], BF, kind="ExternalInput")
    amat_e = nc.dram_tensor("amat", [P, 6 * CH], BF, kind="ExternalInput")
    evwb_e = nc.dram_tensor("evwb", [P, J], BF, kind="ExternalInput")
    wout_e = nc.dram_tensor("wout", [P, KB * NCLS], BF, kind="ExternalInput")
    bkv_e = nc.dram_tensor("bkv", [P, OB], F32, kind="ExternalInput")
    bout_e = nc.dram_tensor("bout", [NCLS, 1], F32, kind="ExternalInput")
    ident_e = nc.dram_tensor("ident", [P, P], BF, kind="ExternalInput")
    out_e = nc.dram_tensor("out", [NCLS, BL], F32, kind="ExternalOutput")

    with tile.TileContext(nc) as tc:
        from contextlib import ExitStack
        with ExitStack() as ctx:
            pers = ctx.enter_context(tc.tile_pool(name="pers", bufs=1))
            # ALL psum pools co-resident (1+2+2+1+2 = 8 banks) so no phase
            # ever waits on a pool-scope boundary; Tile interleaves freely.
            pmisc = ctx.enter_context(tc.tile_pool(name="pmisc", bufs=1, space="PSUM"))
            pkv = ctx.enter_context(tc.tile_pool(name="pkv", bufs=2, space="PSUM"))
            pdot = ctx.enter_context(tc.tile_pool(name="pdot", bufs=1, space="PSUM"))
            ptr = ctx.enter_context(tc.tile_pool(name="ptr", bufs=1, space="PSUM"))
            pfe = ctx.enter_context(tc.tile_pool(name="pfe", bufs=1, space="PSUM"))

            # body emitted `repeat` times for delta-timing benchmarks
            # (tags make repeats share SBUF slots; WAR deps serialize them)
            for _rep in range(repeat):
              # ---- SBUF tiles ----
              bkv_s = pers.tile([P, OB], F32, tag="bkv")
              bout_s = pers.tile([NCLS, 1], F32, tag="bout")
              ident_s = pers.tile([P, P], BF, tag="ident")
              evwb_s = pers.tile([P, J], BF, tag="evwb")
              ones_s = pers.tile([P, 1], BF, tag="ones")
              scratch_s = pers.tile([1, 1], F32, tag="scratch")
              xt_s = pers.tile([P, KB * BL], BF, tag="xt")
              wkv_s = pers.tile([P, OB * KB * P], BF, tag="wkv")
              ekn_s = pers.tile([P, KHB * J], BF, tag="ekn")
              a_s = pers.tile([P, 6 * CH], BF, tag="amat")
              wout_s = pers.tile([P, KB * NCLS], BF, tag="wout")
              kt_s = pers.tile([P, KHB * BL], BF, tag="kt")
              ksq_s = pers.tile([P, KHB * BL], BF, tag="ksq")
              hv_s = pers.tile([P, KHB * BL], BF, tag="hv")
              hfe_s = pers.tile([P, KHB * BL], BF, tag="hfe")
              e_s = pers.tile([P, NB * J], BF, tag="e")
              tmp_s = pers.tile([P, J], BF, tag="tmp")
              u_s = pers.tile([P, NB * J], BF, tag="u")
              ut_s = pers.tile([P, 6 * BL], BF, tag="ut")
              rinv_s = pers.tile([P, NB], F32, tag="rinv")
              rs1_s = pers.tile([P, NB], F32, tag="rs1")
              rs2_s = pers.tile([P, NB], F32, tag="rs2")
              magic_s = pers.tile([P, 1], mybir.dt.int32, tag="magic")
              s_s = pers.tile([P, NB * NCLS], F32, tag="s")
              num_s = pers.tile([P, NB * NCLS], F32, tag="num")
              sinv_s = pers.tile([P, NB * NCLS], F32, tag="sinv")
              t_s = pers.tile([P, NB * NCLS], F32, tag="t")
              g_s = pers.tile([P, NB * NCLS], F32, tag="g")
              gg_s = pers.tile([P, NB], F32, tag="gg")
              ginv_s = pers.tile([P, NB], F32, tag="ginv")
              c1_s = pers.tile([P, NB * NCLS], F32, tag="c1")
              c_s = pers.tile([P, NB * NCLS], F32, tag="c")
              out_sb = pers.tile([NCLS, BL], F32, tag="outsb")

              # ---- DMA schedule ----
              # critical path first on the sync (HWDGE) queue: xt quarters,
              # then k-half weight chunks, ekn (dot), v-half chunks with
              # amat/wout slotted before the last two.
              XQ = 4
              qs = KB * BL // XQ
              nc.sync.dma_start(xt_s[:, 0:qs], xt_e.ap()[:, 0:qs])
              # first weight block right after the first xt quarter so PE can
              # start; remaining xt quarters arrive before k-step 4
              nc.sync.dma_start(wkv_s[:, 0:KB * P], wkv_e.ap()[0])
              for q in range(1, XQ):
                  nc.sync.dma_start(xt_s[:, q * qs:(q + 1) * qs],
                                    xt_e.ap()[:, q * qs:(q + 1) * qs])
              nc.gpsimd.dma_start(bkv_s[:], bkv_e.ap())
              nc.gpsimd.dma_start(bout_s[:], bout_e.ap())
              nc.gpsimd.dma_start(ident_s[:], ident_e.ap())
              nc.gpsimd.dma_start(evwb_s[:], evwb_e.ap())
              nc.vector.memset(ones_s[:], 1.0)
              nc.vector.memset(magic_s[:], 0x5f3759df)

              # dummy Exp as the FIRST ACT op pins the exp table set, which
              # also contains Identity/Relu (all ACT fns used here) -> exactly
              # one table load, executed while PE waits on the first weight DMA
              nc.vector.memset(scratch_s[:], 1.0)
              nc.scalar.activation(scratch_s[:], scratch_s[:], AF.Exp)

              # DMA engines are a shared resource: one consumption-ordered
              # stream beats split queues. ekn/amat/wout go last (consumed at
              # ~30/42/55us, all delivered in time).
              for oj in range(1, OB - 4):
                  nc.sync.dma_start(
                      wkv_s[:, oj * KB * P:(oj + 1) * KB * P], wkv_e.ap()[oj])
              nc.sync.dma_start(ekn_s[:], ekn_e.ap())
              for oj in range(OB - 4, OB - 2):
                  nc.sync.dma_start(
                      wkv_s[:, oj * KB * P:(oj + 1) * KB * P], wkv_e.ap()[oj])
              nc.sync.dma_start(a_s[:], amat_e.ap())
              for oj in range(OB - 2, OB):
                  nc.sync.dma_start(
                      wkv_s[:, oj * KB * P:(oj + 1) * KB * P], wkv_e.ap()[oj])
              nc.sync.dma_start(wout_s[:], wout_e.ap())

              # ---- phase 1: kvT = WkvT^T-blocks @ xT; evict k (+bias) / relu(v+bias) ----
              def kv_block(oj):
                  ps = pkv.tile([P, BL], F32, tag="pkv")
                  base = oj * KB * P
                  for i in range(KB):
                      nc.tensor.matmul(ps[:],
                                       wkv_s[:, base + i * P: base + (i + 1) * P],
                                       xt_s[:, i * BL:(i + 1) * BL],
                                       start=(i == 0), stop=(i == KB - 1))
                  if oj < KHB:
                      sl = slice(oj * BL, (oj + 1) * BL)
                      nc.scalar.activation(kt_s[:, sl], ps[:], AF.Identity,
                                           bias=bkv_s[:, oj:oj + 1])
                      nc.vector.tensor_mul(ksq_s[:, sl], kt_s[:, sl], kt_s[:, sl])
                  else:
                      o2 = oj - KHB
                      nc.scalar.activation(hv_s[:, o2 * BL:(o2 + 1) * BL], ps[:],
                                           AF.Relu, bias=bkv_s[:, oj:oj + 1])

              for oj in range(OB - 4):
                  kv_block(oj)

              # ---- phase 2: sumsq via ones-matmul; rinv = rsqrt on DVE ----
              ps2 = pmisc.tile([P, NB], F32, tag="misc")
              for bc in range(NB):
                  for i in range(KHB):
                      nc.tensor.matmul(ps2[:, bc:bc + 1],
                                       ksq_s[:, i * BL + bc * P: i * BL + bc * P + P],
                                       ones_s[:],
                                       start=(i == 0), stop=(i == KHB - 1))
                  # rinv = rsqrt(sumsq) fully on DVE (magic constant + 2
                  # Newton steps, rel err ~4e-6): no ACT table switches
                  sq = rs1_s[:, bc:bc + 1]
                  nc.vector.tensor_copy(sq, ps2[:, bc:bc + 1])
                  y = rinv_s[:, bc:bc + 1]
                  nc.vector.tensor_scalar(
                      y.bitcast(mybir.dt.int32), sq.bitcast(mybir.dt.int32),
                      1, None, op0=mybir.AluOpType.logical_shift_right)
                  nc.vector.tensor_tensor(
                      out=y.bitcast(mybir.dt.int32), in0=magic_s[:],
                      in1=y.bitcast(mybir.dt.int32),
                      op=mybir.AluOpType.subtract)
                  for _ in range(2):
                      t1 = rs2_s[:, bc:bc + 1]
                      nc.vector.tensor_mul(t1, y, y)
                      nc.vector.tensor_mul(t1, t1, sq)
                      nc.vector.tensor_scalar(t1, t1, -0.5, 1.5,
                                              op0=mybir.AluOpType.mult,
                                              op1=mybir.AluOpType.add)
                      nc.vector.tensor_mul(y, y, t1)

              # ---- phase 3 pieces ----
              def dots(bc):
                  psd = pdot.tile([P, J], F32, tag="pdot")
                  for i in range(KHB):
                      lhs = kt_s[:, i * BL + bc * P: i * BL + bc * P + P]
                      nc.tensor.matmul(psd[:, 0:512], lhs,
                                       ekn_s[:, i * J: i * J + 512],
                                       start=(i == 0), stop=(i == KHB - 1))
                      nc.tensor.matmul(psd[:, 512:J], lhs,
                                       ekn_s[:, i * J + 512:(i + 1) * J],
                                       start=(i == 0), stop=(i == KHB - 1))
                  return psd

              def softmax_chain(bc, psd):
                  e_sl = e_s[:, bc * J:(bc + 1) * J]
                  # exp evict in two halves so the next dots() WAR-waits only
                  # half as long on the psd read
                  nc.scalar.activation(e_sl[:, 0:512], psd[:, 0:512], AF.Exp,
                                       scale=rinv_s[:, bc:bc + 1])
                  nc.scalar.activation(e_sl[:, 512:J], psd[:, 512:J], AF.Exp,
                                       scale=rinv_s[:, bc:bc + 1])
                  e3 = e_sl.rearrange("p (n m) -> p n m", m=M)
                  ncls_sl = slice(bc * NCLS, (bc + 1) * NCLS)
                  s2 = s_s[:, ncls_sl]
                  nc.vector.reduce_sum(s2, e3, axis=mybir.AxisListType.X)
                  nc.vector.tensor_mul(tmp_s[:], e_sl, evwb_s[:])
                  nc.vector.reduce_sum(num_s[:, ncls_sl],
                                       tmp_s[:].rearrange("p (n m) -> p n m", m=M),
                                       axis=mybir.AxisListType.X)
                  nc.vector.reciprocal(sinv_s[:, ncls_sl], s2)
                  nc.vector.tensor_mul(t_s[:, ncls_sl], num_s[:, ncls_sl],
                                       sinv_s[:, ncls_sl])
                  nc.scalar.activation(g_s[:, ncls_sl], t_s[:, ncls_sl], AF.Exp)
                  nc.vector.reduce_sum(gg_s[:, bc:bc + 1], g_s[:, ncls_sl],
                                       axis=mybir.AxisListType.X)
                  nc.vector.reciprocal(ginv_s[:, bc:bc + 1], gg_s[:, bc:bc + 1])
                  nc.vector.tensor_mul(c1_s[:, ncls_sl], g_s[:, ncls_sl],
                                       sinv_s[:, ncls_sl])
                  nc.vector.tensor_scalar_mul(c_s[:, ncls_sl], c1_s[:, ncls_sl],
                                              ginv_s[:, bc:bc + 1])
                  c_b = bass.AP(c_s.tensor, c_s[:, ncls_sl].offset,
                                c_s[:, ncls_sl].ap + [[0, M]])
                  u3 = u_s[:, bc * J:(bc + 1) * J].rearrange("p (n m) -> p n m", m=M)
                  nc.vector.tensor_mul(u3, e3, c_b)

              # ---- phase 4+5 per batch chunk: transpose u, then fE matmuls
              # with 8 accumulators packed into two psum banks; the jb0-2
              # matmuls overlap the group-1 eviction copy on DVE ----
              def transpose_fe(bc):
                  def tgroup(g, grp):
                      pst = ptr.tile([P, 3 * P], BF, tag="ptr")
                      for t, jb in enumerate(grp):
                          w = JBS[jb]
                          nc.tensor.transpose(
                              pst[:w, t * P:(t + 1) * P],
                              u_s[:, bc * J + jb * P: bc * J + jb * P + w],
                              ident_s[:])
                      n = sum(1 for jb in grp if JBS[jb] == P)
                      base = ut_s[:, grp[0] * BL + bc * P: grp[0] * BL + bc * P + P]
                      dst = bass.AP(ut_s.tensor, base.offset,
                                    [base.ap[0], [BL, n], base.ap[1]])
                      nc.vector.tensor_copy(
                          dst, pst[:, 0:n * P].rearrange("p (n q) -> p n q", q=P))
                      if n < len(grp):
                          jb = grp[n]
                          w = JBS[jb]
                          nc.vector.tensor_copy(
                              ut_s[:w, jb * BL + bc * P: jb * BL + bc * P + P],
                              pst[:w, n * P:(n + 1) * P])
                  tgroup(0, (0, 1, 2))
                  tgroup(1, (3, 4, 5))
                  if bc < NB - 1:
                      return
                  # ---- phase 5: fE = A^T-blocks @ uT, full batch width;
                  # relu evicts alternate ACT/DVE so neither queue's
                  # per-instruction overhead rate-limits PE ----
                  for oj in range(KHB):
                      acc = pfe.tile([P, BL], F32, tag=f"pfe{oj % 2}")
                      for jb in range(6):
                          w = JBS[jb]
                          nc.tensor.matmul(
                              acc[:],
                              a_s[:w, jb * CH + oj * P: jb * CH + (oj + 1) * P],
                              ut_s[:w, jb * BL:(jb + 1) * BL],
                              start=(jb == 0), stop=(jb == 5))
                      dst = hfe_s[:, oj * BL:(oj + 1) * BL]
                      if oj % 2 == 0:
                          nc.scalar.activation(dst, acc[:], AF.Relu)
                      else:
                          nc.vector.tensor_scalar_max(dst, acc[:], 0.0)

              # kv blocks 12-15 are PE filler under the two softmax chains
              # (dots/exp/DVE chain latency would otherwise idle PE ~7us)
              psd0 = dots(0)
              softmax_chain(0, psd0)
              psd1 = dots(1)
              softmax_chain(1, psd1)
              kv_block(OB - 4)
              transpose_fe(0)
              kv_block(OB - 3)
              transpose_fe(1)
              kv_block(OB - 2)
              kv_block(OB - 1)

              # ---- phase 6: outT = WoutT^T-blocks @ [hv; hfe]; +bout; DMA out ----
              pso = pmisc.tile([NCLS, BL], F32, tag="misc")
              for i in range(KB):
                  h_s = hv_s if i < KHB else hfe_s
                  ii = i % KHB
                  nc.tensor.matmul(pso[:], wout_s[:, i * NCLS:(i + 1) * NCLS],
                                   h_s[:, ii * BL:(ii + 1) * BL],
                                   start=(i == 0), stop=(i == KB - 1))
              nc.vector.tensor_scalar_add(out_sb[:], pso[:], bout_s[:, 0:1])
              nc.sync.dma_start(out_e.ap(), out_sb[:])

    nc.compile()
    return nc


def host_prep(x, static_feat, Wk, bk, Wv, bv, WEk, bEk, WEv, bEv, Ww, bw,
              Wout, bout):
    """Host-side fp32 precompute + per-core input maps."""
    EPS = 1e-8
    f32 = np.float32
    x = np.asarray(x, f32)
    static_feat = np.asarray(static_feat, f32)

    Ek = np.einsum('oc,ncm->nom', np.asarray(WEk, f32), static_feat,
                   optimize=True) + np.asarray(bEk, f32)[None, :, None]
    Ev = np.einsum('oc,ncm->nom', np.asarray(WEv, f32), static_feat,
                   optimize=True) + np.asarray(bEv, f32)[None, :, None]
    Ekn = Ek / np.maximum(np.linalg.norm(Ek, axis=1, keepdims=True), EPS)
    Ekn_mat = Ekn.transpose(1, 0, 2).reshape(CH, J)          # [CH, 672]
    A_mat = Ev.transpose(0, 2, 1).reshape(J, CH)             # [672, CH]
    evwb = np.einsum('nom,o->nm', Ev, np.asarray(Ww, f32)[0]).reshape(J)

    WkvT = np.concatenate([np.asarray(Wk, f32), np.asarray(Wv, f32)], axis=0).T
    bkv = np.concatenate([np.asarray(bk, f32), np.asarray(bv, f32)])
    xT = np.ascontiguousarray(x[:, -1, :].T)                 # [CIN, B]

    # [OB, P, KB*P]: per-o-chunk, per-partition-linear
    wkv_h = np.ascontiguousarray(
        WkvT.reshape(KB, P, OB, P).transpose(2, 1, 0, 3).reshape(
            OB, P, KB * P)).astype(bf16)
    ekn_h = np.ascontiguousarray(
        Ekn_mat.reshape(KHB, P, J).transpose(1, 0, 2).reshape(
            P, KHB * J)).astype(bf16)
    a_pad = np.zeros((6 * P, CH), np.float32)
    a_pad[:J] = A_mat
    amat_h = np.ascontiguousarray(
        a_pad.reshape(6, P, CH).transpose(1, 0, 2).reshape(P, 6 * CH)).astype(bf16)
    evwb_h = np.ascontiguousarray(
        np.broadcast_to(evwb.astype(bf16)[None, :], (P, J)))
    wout_h = np.ascontiguousarray(
        np.asarray(Wout, f32).T.reshape(KB, P, NCLS).transpose(1, 0, 2).reshape(
            P, KB * NCLS)).astype(bf16)
    bkv_h = np.ascontiguousarray(bkv.reshape(OB, P).T)
    bout_h = np.asarray(bout, f32).reshape(NCLS, 1)
    ident_h = np.eye(P, dtype=bf16)

    shared = dict(wkv=wkv_h, ekn=ekn_h, amat=amat_h, evwb=evwb_h,
                  wout=wout_h, bkv=bkv_h, bout=bout_h, ident=ident_h)
    in_maps = []
    for c in range(NCORES):
        xt_h = np.ascontiguousarray(
            xT[:, c * BL:(c + 1) * BL].reshape(KB, P, BL).transpose(1, 0, 2)
            .reshape(P, KB * BL)).astype(bf16)
        in_maps.append(dict(xt=xt_h, **shared))
    return in_maps


_NC_CACHE = {}


def get_nc(debug=False, repeat=1):
    key = (debug, repeat)
    if key not in _NC_CACHE:
        _NC_CACHE[key] = build_nc(debug=debug, repeat=repeat)
    return _NC_CACHE[key]


def kernel(**inputs) -> np.ndarray:
    nc = get_nc()
    in_maps = host_prep(**inputs)
    res = run_bass_kernel_spmd(nc, in_maps, list(range(NCORES)))
    out = np.empty((B, NCLS, 1), dtype=np.float32)
    for c in range(NCORES):
        out[c * BL:(c + 1) * BL, :, 0] = res.results[c]["out"].T
    return out



# revision 12
# speedup vs baseline: 1.3814x; 1.3814x over previous
"""Trainium2 Bass kernel for nn_Colar_static (retrieval_knn).

Sharding: data-parallel over batch B=2048 across 8 NeuronCores (256 rows each).
Static exemplar banks and weights are precomputed/reshaped on host and
replicated to all cores.

fp8 strategy (DoubleRow perf mode = 0.5 cycles/row, one instr per K-tile
PAIR). All quantization is host-side e4m3 with power-of-2 scales folded
into PSUM evictions:
  - k-half:  k*64 = x1 @ (64*Wk)_q             (plain fp8; error is diluted
    through the exemplar softmax, verified numerically at ~3e-4 effect)
  - v-half:  v*64 = x1@V1 + x2@V1 + x1@V2 where V = 64*Wv, V1=q(V),
    V2=q(V-V1), x1=q(x), x2=q(x-x1)  (residual-compensated fp8: same HBM
    bytes as bf16 but 2.7x fewer PE cycles; error ~0.5% of v)
  - dots:    kt_fp8 @ (32*Ekn)_q; the 1/32 is folded into rinv by summing
    ksq against ones=1024 (rinv = rsqrt(1024*|k|^2) = (1/32)/|k|)
  - fE:      u_fp8 (c scaled by 256) @ A_q; relu evict scales by 1/256
  - out:     stays bf16 (fp8 here measurably breaks the 2e-2 tolerance)

Per-core pipeline (j = flattened (class, exemplar) = 21*32 = 672):
  1. kT,vT      = DoubleRow fp8 matmuls, K=2048      (PE)
  2. sumsq[b]   = ones(1024)-matmul over kT^2; rinv = rsqrt on DVE
  3. dot[b,j]   = kT-pairs^T @ Ekn (DoubleRow)       (PE, K=1024)
  4. e = exp(rinv*dot)  (ACT per-partition scale)
  5. blockwise softmax chain on DVE (class softmax; bw cancels)
  6. u[b,j] = e * c*256 (fp8)                        (DVE)
  7. uT = PE-transpose(u) (fp8)
  8. fE_T[o,b]  = A-pairs^T @ uT (DoubleRow), relu*(1/256) evict
  9. outT[cls,b]= WoutT^T-blocks @ [relu(vT); relu(fE_T)] in bf16
 10. out = psum + bout -> DMA

Host gathers the 8 [21,256] results into [2048, 21, 1].
"""

import numpy as np
import ml_dtypes

import concourse.bass as bass
import concourse.bacc as bacc
import concourse.mybir as mybir
import concourse.tile as tile
from concourse.bass_utils import run_bass_kernel_spmd

AF = mybir.ActivationFunctionType
BF = mybir.dt.bfloat16
F8 = mybir.dt.float8e4
F32 = mybir.dt.float32
bf16 = ml_dtypes.bfloat16
f8 = ml_dtypes.float8_e4m3
PM = mybir.MatmulPerfMode.DoubleRow

# Problem constants (hardcoded; kernel.py must be self-contained)
B, T, CIN, CH, M, NCLS = 2048, 8, 2048, 1024, 32, 21
NCORES = 8
BL = B // NCORES          # 256 batch rows per core
J = NCLS * M              # 672
P = 128
KB = CIN // P             # 16 contraction blocks for kv
KP = KB // 2              # 8 DoubleRow k-tile pairs
OB = 2 * CH // P          # 16 output-channel blocks for kv
KHB = CH // P             # 8 blocks of k/v half
NB = BL // P              # 2 batch chunks of 128
WSC = 64.0                # weight scale for Wk/Wv fp8
ESC = 32.0                # Ekn fp8 scale
USC = 256.0               # u fp8 scale


def pair(ap_base, stride):
    """[p, N] AP -> [p, 2, N] AP whose middle dim steps by `stride` elems."""
    return bass.AP(ap_base.tensor, ap_base.offset,
                   [ap_base.ap[0], [stride, 2], ap_base.ap[-1]])


def build_nc(debug=False, repeat=1):
    nc = bacc.Bacc("TRN2", target_bir_lowering=False, debug=debug,
                   num_devices=NCORES)

    # all inputs are shipped in the exact per-partition SBUF layout so every
    # DMA is a plain [128, N]-contiguous copy (max DMA efficiency)
    x1_e = nc.dram_tensor("x1", [P, KB * BL], F8, kind="ExternalInput")
    x2_e = nc.dram_tensor("x2", [P, KB * BL], F8, kind="ExternalInput")
    wk1_e = nc.dram_tensor("wk1", [P, KHB * KB * P], F8, kind="ExternalInput")
    wv1_e = nc.dram_tensor("wv1", [P, KHB * KB * P], F8, kind="ExternalInput")
    wv2_e = nc.dram_tensor("wv2", [P, KHB * KB * P], F8, kind="ExternalInput")
    ekn_e = nc.dram_tensor("ekn", [P, KHB * J], F8, kind="ExternalInput")
    amat_e = nc.dram_tensor("amat", [P, 6 * CH], F8, kind="ExternalInput")
    evwb_e = nc.dram_tensor("evwb", [P, J], BF, kind="ExternalInput")
    wout_e = nc.dram_tensor("wout", [P, KB * NCLS], BF, kind="ExternalInput")
    bkv_e = nc.dram_tensor("bkv", [P, OB], F32, kind="ExternalInput")
    bout_e = nc.dram_tensor("bout", [NCLS, 1], F32, kind="ExternalInput")
    ident_e = nc.dram_tensor("ident", [P, P], BF, kind="ExternalInput")
    out_e = nc.dram_tensor("out", [NCLS, BL], F32, kind="ExternalOutput")

    with tile.TileContext(nc) as tc:
        from contextlib import ExitStack
        with ExitStack() as ctx:
            pers = ctx.enter_context(tc.tile_pool(name="pers", bufs=1))
            # ALL psum pools co-resident so no phase waits on a pool-scope
            # boundary; Tile interleaves freely.
            pmisc = ctx.enter_context(tc.tile_pool(name="pmisc", bufs=1, space="PSUM"))
            pkv = ctx.enter_context(tc.tile_pool(name="pkv", bufs=2, space="PSUM"))
            pdot = ctx.enter_context(tc.tile_pool(name="pdot", bufs=1, space="PSUM"))
            ptr = ctx.enter_context(tc.tile_pool(name="ptr", bufs=1, space="PSUM"))
            pfe = ctx.enter_context(tc.tile_pool(name="pfe", bufs=1, space="PSUM"))

            # body emitted `repeat` times for delta-timing benchmarks
            for _rep in range(repeat):
              # ---- SBUF tiles ----
              bkv_s = pers.tile([P, OB], F32, tag="bkv")
              bout_s = pers.tile([NCLS, 1], F32, tag="bout")
              ident_s = pers.tile([P, P], BF, tag="ident")
              evwb_s = pers.tile([P, J], BF, tag="evwb")
              ones_s = pers.tile([P, 1], BF, tag="ones")
              scratch_s = pers.tile([1, 1], F32, tag="scratch")
              x1_s = pers.tile([P, KB * BL], F8, tag="x1")
              x2_s = pers.tile([P, KB * BL], F8, tag="x2")
              wk1_s = pers.tile([P, KHB * KB * P], F8, tag="wk1")
              wv1_s = pers.tile([P, KHB * KB * P], F8, tag="wv1")
              wv2_s = pers.tile([P, KHB * KB * P], F8, tag="wv2")
              ekn_s = pers.tile([P, KHB * J], F8, tag="ekn")
              a_s = pers.tile([P, 6 * CH], F8, tag="amat")
              wout_s = pers.tile([P, KB * NCLS], BF, tag="wout")
              kt_s = pers.tile([P, KHB * BL], F8, tag="kt")
              ksq_s = pers.tile([P, KHB * BL], BF, tag="ksq")
              hv_s = pers.tile([P, KHB * BL], BF, tag="hv")
              hfe_s = pers.tile([P, KHB * BL], BF, tag="hfe")
              e_s = pers.tile([P, NB * J], BF, tag="e")
              tmp_s = pers.tile([P, J], BF, tag="tmp")
              u_s = pers.tile([P, NB * J], BF, tag="u")
              ut_s = pers.tile([P, 6 * BL], F8, tag="ut")
              rinv_s = pers.tile([P, NB], F32, tag="rinv")
              rs1_s = pers.tile([P, NB], F32, tag="rs1")
              rs2_s = pers.tile([P, NB], F32, tag="rs2")
              magic_s = pers.tile([P, 1], mybir.dt.int32, tag="magic")
              s_s = pers.tile([P, NB * NCLS], F32, tag="s")
              num_s = pers.tile([P, NB * NCLS], F32, tag="num")
              sinv_s = pers.tile([P, NB * NCLS], F32, tag="sinv")
              t_s = pers.tile([P, NB * NCLS], F32, tag="t")
              g_s = pers.tile([P, NB * NCLS], F32, tag="g")
              gg_s = pers.tile([P, NB], F32, tag="gg")
              ginv_s = pers.tile([P, NB], F32, tag="ginv")
              c1_s = pers.tile([P, NB * NCLS], F32, tag="c1")
              c_s = pers.tile([P, NB * NCLS], F32, tag="c")
              out_sb = pers.tile([NCLS, BL], F32, tag="outsb")

              # ---- DMA schedule ----
              # One consumption-ordered HWDGE stream (DMA engines are a
              # globally shared resource; order = just-in-time arrival).
              WCH = KB * P  # 2048 elems per weight oj-block
              XQ = 4
              qs = KB * BL // XQ
              nc.sync.dma_start(x1_s[:, 0:qs], x1_e.ap()[:, 0:qs])
              nc.sync.dma_start(wk1_s[:, 0:2 * WCH], wk1_e.ap()[:, 0:2 * WCH])
              for q in range(1, XQ):
                  nc.sync.dma_start(x1_s[:, q * qs:(q + 1) * qs],
                                    x1_e.ap()[:, q * qs:(q + 1) * qs])
              # small constants ride the SWDGE (Pool) path off the HWDGE queue
              nc.gpsimd.dma_start(bkv_s[:], bkv_e.ap())
              nc.gpsimd.dma_start(bout_s[:], bout_e.ap())
              nc.gpsimd.dma_start(ident_s[:], ident_e.ap())
              nc.gpsimd.dma_start(evwb_s[:], evwb_e.ap())
              nc.vector.memset(ones_s[:], 1024.0)
              nc.vector.memset(magic_s[:], 0x5f3759df)
              # fE reads ut pairs through zero A-columns for the j-padding;
              # zero the jb5 block so stray NaN bytes in never-written
              # partitions can't poison the psum through 0*NaN. (Full 128
              # partitions: a memset may not start at partition 32.) The
              # transposes later overwrite rows 0:32 with real data.
              nc.vector.memset(ut_s[:, 5 * BL:6 * BL], 0.0)

              # dummy Exp as the FIRST ACT op pins the exp table set, which
              # also contains Identity/Relu (all ACT fns used here) -> exactly
              # one table load, executed while PE waits on the first weight DMA
              nc.vector.memset(scratch_s[:], 1.0)
              nc.scalar.activation(scratch_s[:], scratch_s[:], AF.Exp)

              def wdma(sb, e, c0, nb):
                  nc.sync.dma_start(sb[:, c0 * WCH:(c0 + nb) * WCH],
                                    e.ap()[:, c0 * WCH:(c0 + nb) * WCH])
              for cc in range(1, 4):
                  wdma(wk1_s, wk1_e, 2 * cc, 2)
              nc.sync.dma_start(ekn_s[:], ekn_e.ap())
              nc.sync.dma_start(x2_s[:], x2_e.ap())
              for cc in range(2):
                  wdma(wv1_s, wv1_e, 2 * cc, 2)
                  wdma(wv2_s, wv2_e, 2 * cc, 2)
              nc.sync.dma_start(a_s[:], amat_e.ap())
              for cc in range(2, 4):
                  wdma(wv1_s, wv1_e, 2 * cc, 2)
                  if cc == 2:
                      wdma(wv2_s, wv2_e, 4, 2)
              nc.sync.dma_start(wout_s[:], wout_e.ap())
              wdma(wv2_s, wv2_e, 6, 1)
              wdma(wv2_s, wv2_e, 7, 1)

              # ---- phase 1: kv via DoubleRow fp8; evict k (+bias) / relu(v) ----
              def k_block(oj):
                  ps = pkv.tile([P, BL], F32, tag="pkv")
                  base = oj * WCH
                  for i in range(KP):
                      nc.tensor.matmul(
                          ps[:],
                          wk1_s[:, base + 2 * i * P: base + 2 * (i + 1) * P]
                          .rearrange("p (two f) -> p two f", two=2),
                          x1_s[:, 2 * i * BL:2 * (i + 1) * BL]
                          .rearrange("p (two f) -> p two f", two=2),
                          start=(i == 0), stop=(i == KP - 1), perf_mode=PM)
                  sl = slice(oj * BL, (oj + 1) * BL)
                  nc.scalar.activation(kt_s[:, sl], ps[:], AF.Identity,
                                       bias=bkv_s[:, oj:oj + 1], scale=1.0 / WSC)
                  nc.vector.tensor_mul(ksq_s[:, sl], kt_s[:, sl], kt_s[:, sl])

              def v_block(o2):
                  # accumulate x1@V1 + x2@V1 + x1@V2 at psum scale 64
                  ps = pkv.tile([P, BL], F32, tag="pkv")
                  base = o2 * WCH
                  first = True
                  for xs, ws in ((x1_s, wv1_s), (x2_s, wv1_s), (x1_s, wv2_s)):
                      for i in range(KP):
                          nc.tensor.matmul(
                              ps[:],
                              ws[:, base + 2 * i * P: base + 2 * (i + 1) * P]
                              .rearrange("p (two f) -> p two f", two=2),
                              xs[:, 2 * i * BL:2 * (i + 1) * BL]
                              .rearrange("p (two f) -> p two f", two=2),
                              start=first,
                              stop=(xs is x1_s and ws is wv2_s and i == KP - 1),
                              perf_mode=PM)
                          first = False
                  nc.scalar.activation(hv_s[:, o2 * BL:(o2 + 1) * BL], ps[:],
                                       AF.Relu, bias=bkv_s[:, KHB + o2:KHB + o2 + 1],
                                       scale=1.0 / WSC)

              for oj in range(KHB):
                  k_block(oj)

              # ---- phase 2: sumsq via ones(1024)-matmul; rinv = rsqrt on DVE ----
              ps2 = pmisc.tile([P, NB], F32, tag="misc")
              for bc in range(NB):
                  for i in range(KHB):
                      nc.tensor.matmul(ps2[:, bc:bc + 1],
                                       ksq_s[:, i * BL + bc * P: i * BL + bc * P + P],
                                       ones_s[:],
                                       start=(i == 0), stop=(i == KHB - 1))
                  # rinv = rsqrt(1024*sumsq) fully on DVE (magic constant + 2
                  # Newton steps, rel err ~4e-6): no ACT table switches
                  sq = rs1_s[:, bc:bc + 1]
                  nc.vector.tensor_copy(sq, ps2[:, bc:bc + 1])
                  y = rinv_s[:, bc:bc + 1]
                  nc.vector.tensor_scalar(
                      y.bitcast(mybir.dt.int32), sq.bitcast(mybir.dt.int32),
                      1, None, op0=mybir.AluOpType.logical_shift_right)
                  nc.vector.tensor_tensor(
                      out=y.bitcast(mybir.dt.int32), in0=magic_s[:],
                      in1=y.bitcast(mybir.dt.int32),
                      op=mybir.AluOpType.subtract)
                  for _ in range(2):
                      t1 = rs2_s[:, bc:bc + 1]
                      nc.vector.tensor_mul(t1, y, y)
                      nc.vector.tensor_mul(t1, t1, sq)
                      nc.vector.tensor_scalar(t1, t1, -0.5, 1.5,
                                              op0=mybir.AluOpType.mult,
                                              op1=mybir.AluOpType.add)
                      nc.vector.tensor_mul(y, y, t1)

              # ---- phase 3 pieces ----
              def dots(bc):
                  psd = pdot.tile([P, J], F32, tag="pdot")
                  for i in range(KP // 2):
                      lhs = pair(kt_s[:, 2 * i * BL + bc * P:
                                      2 * i * BL + bc * P + P], BL)
                      for j0, j1 in ((0, 256), (256, 512), (512, J)):
                          nc.tensor.matmul(
                              psd[:, j0:j1],
                              lhs,
                              pair(ekn_s[:, 2 * i * J + j0: 2 * i * J + j1], J),
                              start=(i == 0), stop=(i == KP // 2 - 1),
                              perf_mode=PM)
                  return psd

              def softmax_chain(bc, psd):
                  e_sl = e_s[:, bc * J:(bc + 1) * J]
                  # exp evict in two halves so the next dots() WAR-waits only
                  # half as long on the psd read
                  nc.scalar.activation(e_sl[:, 0:512], psd[:, 0:512], AF.Exp,
                                       scale=rinv_s[:, bc:bc + 1])
                  nc.scalar.activation(e_sl[:, 512:J], psd[:, 512:J], AF.Exp,
                                       scale=rinv_s[:, bc:bc + 1])
                  e3 = e_sl.rearrange("p (n m) -> p n m", m=M)
                  ncls_sl = slice(bc * NCLS, (bc + 1) * NCLS)
                  s2 = s_s[:, ncls_sl]
                  nc.vector.reduce_sum(s2, e3, axis=mybir.AxisListType.X)
                  nc.vector.tensor_mul(tmp_s[:], e_sl, evwb_s[:])
                  nc.vector.reduce_sum(num_s[:, ncls_sl],
                                       tmp_s[:].rearrange("p (n m) -> p n m", m=M),
                                       axis=mybir.AxisListType.X)
                  nc.vector.reciprocal(sinv_s[:, ncls_sl], s2)
                  nc.vector.tensor_mul(t_s[:, ncls_sl], num_s[:, ncls_sl],
                                       sinv_s[:, ncls_sl])
                  nc.scalar.activation(g_s[:, ncls_sl], t_s[:, ncls_sl], AF.Exp)
                  nc.vector.reduce_sum(gg_s[:, bc:bc + 1], g_s[:, ncls_sl],
                                       axis=mybir.AxisListType.X)
                  nc.vector.reciprocal(ginv_s[:, bc:bc + 1], gg_s[:, bc:bc + 1])
                  nc.vector.tensor_mul(c1_s[:, ncls_sl], g_s[:, ncls_sl],
                                       sinv_s[:, ncls_sl])
                  # c = (fw/S) * 256 so u lands in fp8's normal range
                  nc.vector.tensor_scalar(c_s[:, ncls_sl], c1_s[:, ncls_sl],
                                          ginv_s[:, bc:bc + 1], USC,
                                          op0=mybir.AluOpType.mult,
                                          op1=mybir.AluOpType.mult)
                  c_b = bass.AP(c_s.tensor, c_s[:, ncls_sl].offset,
                                c_s[:, ncls_sl].ap + [[0, M]])
                  u3 = u_s[:, bc * J:(bc + 1) * J].rearrange("p (n m) -> p n m", m=M)
                  nc.vector.tensor_mul(u3, e3, c_b)

              # ---- phase 4+5 per batch chunk: transpose u (fp8), then fE
              # DoubleRow matmuls; relu evicts alternate ACT/DVE ----
              JBS = [P] * 5 + [J - 5 * P]
              def transpose_fe(bc):
                  # u transposes run in bf16 (fp8 PE-transpose needs stride-2
                  # psum writes); the DVE eviction copy converts to fp8 ut.
                  def tgroup(g, grp):
                      pst = ptr.tile([P, 3 * P], BF, tag="ptr")
                      for t, jb in enumerate(grp):
                          w = JBS[jb]
                          nc.tensor.transpose(
                              pst[:w, t * P:(t + 1) * P],
                              u_s[:, bc * J + jb * P: bc * J + jb * P + w],
                              ident_s[:])
                      n = sum(1 for jb in grp if JBS[jb] == P)
                      base = ut_s[:, grp[0] * BL + bc * P: grp[0] * BL + bc * P + P]
                      dst = bass.AP(ut_s.tensor, base.offset,
                                    [base.ap[0], [BL, n], base.ap[1]])
                      nc.vector.tensor_copy(
                          dst, pst[:, 0:n * P].rearrange("p (n q) -> p n q", q=P))
                      if n < len(grp):
                          jb = grp[n]
                          w = JBS[jb]
                          nc.vector.tensor_copy(
                              ut_s[:w, jb * BL + bc * P: jb * BL + bc * P + P],
                              pst[:w, n * P:(n + 1) * P])
                  tgroup(0, (0, 1, 2))
                  tgroup(1, (3, 4, 5))
                  if bc < NB - 1:
                      return
                  # ---- phase 5: fE = A-pairs^T @ uT-pairs (DoubleRow),
                  # relu*(1/USC) evicts alternate ACT/DVE ----
                  for oj in range(KHB):
                      acc = pfe.tile([P, BL], F32, tag=f"pfe{oj % 2}")
                      for jp in range(3):
                          nc.tensor.matmul(
                              acc[:],
                              pair(a_s[:, 2 * jp * CH + oj * P:
                                       2 * jp * CH + (oj + 1) * P], CH),
                              pair(ut_s[:, 2 * jp * BL:(2 * jp + 1) * BL], BL),
                              start=(jp == 0), stop=(jp == 2), perf_mode=PM)
                      dst = hfe_s[:, oj * BL:(oj + 1) * BL]
                      if oj % 2 == 0:
                          nc.scalar.activation(dst, acc[:], AF.Relu,
                                               scale=1.0 / USC)
                      else:
                          nc.vector.tensor_scalar(dst, acc[:], 1.0 / USC, 0.0,
                                                  op0=mybir.AluOpType.mult,
                                                  op1=mybir.AluOpType.max)

              # v blocks are PE filler under the two softmax chains
              psd0 = dots(0)
              softmax_chain(0, psd0)
              v_block(0)
              psd1 = dots(1)
              softmax_chain(1, psd1)
              v_block(1)
              transpose_fe(0)
              v_block(2)
              transpose_fe(1)
              for o2 in range(3, KHB):
                  v_block(o2)

              # ---- phase 6: outT = WoutT^T-blocks @ [hv; hfe] in bf16;
              # k-block 7 (last hv block) goes last so the tail after the
              # final wv2 DMA is minimal ----
              pso = pmisc.tile([NCLS, BL], F32, tag="misc")
              korder = [0, 1, 2, 3, 4, 5, 6] + list(range(KHB, KB)) + [7]
              for n, i in enumerate(korder):
                  h_s = hv_s if i < KHB else hfe_s
                  ii = i % KHB
                  nc.tensor.matmul(pso[:], wout_s[:, i * NCLS:(i + 1) * NCLS],
                                   h_s[:, ii * BL:(ii + 1) * BL],
                                   start=(n == 0), stop=(n == KB - 1))
              nc.vector.tensor_scalar_add(out_sb[:], pso[:], bout_s[:, 0:1])
              nc.sync.dma_start(out_e.ap(), out_sb[:])

    nc.compile()
    return nc


def host_prep(x, static_feat, Wk, bk, Wv, bv, WEk, bEk, WEv, bEv, Ww, bw,
              Wout, bout):
    """Host-side fp32 precompute + per-core input maps."""
    EPS = 1e-8
    f32 = np.float32
    x = np.asarray(x, f32)
    static_feat = np.asarray(static_feat, f32)

    Ek = np.einsum('oc,ncm->nom', np.asarray(WEk, f32), static_feat,
                   optimize=True) + np.asarray(bEk, f32)[None, :, None]
    Ev = np.einsum('oc,ncm->nom', np.asarray(WEv, f32), static_feat,
                   optimize=True) + np.asarray(bEv, f32)[None, :, None]
    Ekn = Ek / np.maximum(np.linalg.norm(Ek, axis=1, keepdims=True), EPS)
    Ekn_mat = Ekn.transpose(1, 0, 2).reshape(CH, J)          # [CH, 672]
    A_mat = Ev.transpose(0, 2, 1).reshape(J, CH)             # [672, CH]
    evwb = np.einsum('nom,o->nm', Ev, np.asarray(Ww, f32)[0]).reshape(J)

    def wlayout(WT):
        """[CIN, CH] fp32 -> [P, KHB*KB*P]: per-partition, oj-block-major."""
        return WT.reshape(KB, P, KHB, P).transpose(1, 2, 0, 3).reshape(
            P, KHB * KB * P)

    Ak = WSC * np.asarray(Wk, f32).T                         # [CIN, CH]
    Av = WSC * np.asarray(Wv, f32).T
    Ak1 = Ak.astype(f8)
    Av1 = Av.astype(f8)
    Av2 = (Av - Av1.astype(f32)).astype(f8)
    wk1_h = np.ascontiguousarray(wlayout(Ak1.astype(f32)).astype(f8))
    wv1_h = np.ascontiguousarray(wlayout(Av1.astype(f32)).astype(f8))
    wv2_h = np.ascontiguousarray(wlayout(Av2.astype(f32)).astype(f8))

    ekn_h = np.ascontiguousarray(
        (ESC * Ekn_mat).reshape(KHB, P, J).transpose(1, 0, 2).reshape(
            P, KHB * J)).astype(f8)
    a_pad = np.zeros((6 * P, CH), np.float32)
    a_pad[:J] = A_mat
    amat_h = np.ascontiguousarray(
        a_pad.reshape(6, P, CH).transpose(1, 0, 2).reshape(P, 6 * CH)).astype(f8)
    evwb_h = np.ascontiguousarray(
        np.broadcast_to(evwb.astype(bf16)[None, :], (P, J)))
    wout_h = np.ascontiguousarray(
        np.asarray(Wout, f32).T.reshape(KB, P, NCLS).transpose(1, 0, 2).reshape(
            P, KB * NCLS)).astype(bf16)
    bkv = np.concatenate([np.asarray(bk, f32), np.asarray(bv, f32)])
    bkv_h = np.ascontiguousarray(bkv.reshape(OB, P).T)
    bout_h = np.asarray(bout, f32).reshape(NCLS, 1)
    ident_h = np.eye(P, dtype=bf16)

    shared = dict(wk1=wk1_h, wv1=wv1_h, wv2=wv2_h, ekn=ekn_h, amat=amat_h,
                  evwb=evwb_h, wout=wout_h, bkv=bkv_h, bout=bout_h,
                  ident=ident_h)
    xT = np.ascontiguousarray(x[:, -1, :].T)                 # [CIN, B]
    in_maps = []
    for c in range(NCORES):
        xc = xT[:, c * BL:(c + 1) * BL]
        x1 = xc.astype(f8)
        x2 = (xc - x1.astype(f32)).astype(f8)
        def xlayout(a):
            return np.ascontiguousarray(
                a.astype(f32).reshape(KB, P, BL).transpose(1, 0, 2)
                .reshape(P, KB * BL)).astype(f8)
        in_maps.append(dict(x1=xlayout(x1), x2=xlayout(x2), **shared))
    return in_maps


_NC_CACHE = {}


def get_nc(debug=False, repeat=1):
    key = (debug, repeat)
    if key not in _NC_CACHE:
        _NC_CACHE[key] = build_nc(debug=debug, repeat=repeat)
    return _NC_CACHE[key]


def kernel(**inputs) -> np.ndarray:
    nc = get_nc()
    in_maps = host_prep(**inputs)
    res = run_bass_kernel_spmd(nc, in_maps, list(range(NCORES)))
    out = np.empty((B, NCLS, 1), dtype=np.float32)
    for c in range(NCORES):
        out[c * BL:(c + 1) * BL, :, 0] = res.results[c]["out"].T
    return out
